# revision 60
# baseline (speedup 1.0000x reference)
"""CapsNet dynamic-routing kernel for Trainium2 (8 NeuronCores, SPMD).

Math (see reference):
  u_hat[j,b,k,u] = sum_d W[j,k,d,u] * x[b,k,d]
  for j in 0..9:  (sequential, b_IJ carried)
    3 routing iterations:
      c_k      = softmax(b_IJ, axis=1)[:, j]
      s[b,u]   = sum_k c_k u_hat[j,b,k,u]
      v        = squash(s)
      agree[k] = sum_{b,u} u_hat[j,b,k,u] v[b,u]   (sum over FULL batch)
      b_IJ[:, j] += agree
  out[b,j,u] = v (last iteration of each j)

Distribution: data-parallel over batch (64 per core).  The only cross-core
quantity is agree -> AllReduce[1152] each routing iteration (29 total; the
final (j=9,t=2) update is dead and skipped).

All matmul operands are bf16 (fp32 matmuls double-pass on the PE,
fp32_mode=LOW_HIGH); accumulation is fp32 in PSUM, and all routing state
(e3/softmax/squash/AllReduce) stays fp32.

Per-core layouts (G=72 groups of 16 k).  DMA partition ranges must be
contiguous, which forces two partition orders:
  k-major  p = khat*8 + d   : wbd (block-diag W_j band writes), x_a
  d-major  p = d*16 + khat  : wj, cW, c, e3, agr (replicate writes), x_b
  x_a/x_b SBUF [128, G, 64] bf16     : x[b, g*16+khat, d] in each order
  wbd     SBUF [128, G, 256] bf16    : block-diag W_j; col c=(khat',u)
  u_hat A SBUF [64=b, G, 16=khat, 16=u] bf16  (from PE matmuls via PSUM)
  e3      SBUF [128, G, 10] fp32     : exp(b_IJ); softmax needs no
                                       max-subtraction (|b_IJ| < 5 here)
s-matvec avoids u_hat entirely:  s = sum_g x_b[:,g,:]^T @ (c*W_j)[:,g,:]
agree: 48 accumulating matmuls  lhsT=v_bf[:,u], rhs=A[:, g-third, :, u].
u_hat for capsule j+1 is emitted interleaved into j's routing iterations
so the PE has work during the AllReduce stalls (HAM stays warm).
"""

import numpy as np
import ml_dtypes

import concourse.bass as bass
import concourse.bacc as bacc
import concourse.mybir as mybir
import concourse.tile as tile
from concourse.tile import add_dep_helper
from concourse import bass_utils

F32 = mybir.dt.float32
BF16 = mybir.dt.bfloat16
AF = mybir.ActivationFunctionType
ALU = mybir.AluOpType

J = 10        # output capsules
K = 1152      # input capsules
D = 8         # in dim
U = 16        # out dim
B = 512       # batch
N_CORES = 8
ITERS = 3
EPS = 1e-7
G = K // 16   # 72 groups of 16 k


def capsnet_body(tc, nc, x_dram, w_dram, out_dram, replica_groups, b_local):
    """Emit the per-core program. x [2,128,G,b] bf16, w [J,K,D,U] bf16,
    out [b, J, U] fp32."""
    from contextlib import ExitStack
    es = ExitStack()
    p_const = es.enter_context(tc.tile_pool(name="const", bufs=1))
    p_uhat = es.enter_context(tc.tile_pool(name="uhat", bufs=2))
    p_wj = es.enter_context(tc.tile_pool(name="wj", bufs=2))
    p_cw = es.enter_context(tc.tile_pool(name="cw", bufs=2))
    p_small = es.enter_context(tc.tile_pool(name="small", bufs=4))
    p_v = es.enter_context(tc.tile_pool(name="vpool", bufs=2))
    p_agr = es.enter_context(tc.tile_pool(name="agr", bufs=2))
    p_ps_uh = es.enter_context(tc.tile_pool(name="ps_uh", bufs=2, space="PSUM"))
    p_ps_s = es.enter_context(tc.tile_pool(name="ps_s", bufs=2, space="PSUM"))
    p_ps_a = es.enter_context(tc.tile_pool(name="ps_a", bufs=1, space="PSUM"))
    p_dram = es.enter_context(tc.tile_pool(name="dram", bufs=4, space="DRAM"))

    # ---- persistent tiles
    x_a = p_const.tile([128, G, b_local], BF16, tag="x_a")
    x_b = p_const.tile([128, G, b_local], BF16, tag="x_b")
    wbd_a = p_const.tile([128, G, 256], BF16, tag="wbd_a")
    wbd_b = p_const.tile([128, G, 256], BF16, tag="wbd_b")
    e3 = p_const.tile([128, G, J], F32, tag="e3")
    den = p_const.tile([128, G], F32, tag="den")  # sum_j e3, kept incrementally
    wbds = [wbd_a, wbd_b]

    nc.gpsimd.dma_start(x_a[:], x_dram.ap()[0])
    nc.gpsimd.dma_start(x_b[:], x_dram.ap()[1])
    nc.vector.memset(wbd_a[:], 0.0)
    nc.vector.memset(wbd_b[:], 0.0)
    nc.vector.memset(e3[:], 1.0)
    nc.vector.memset(den[:], float(J))

    wj_tiles = {}
    A_tiles = {}
    dma_engines = [nc.gpsimd, nc.scalar]

    def emit_w_loads(j):
        """wj DMA + block-diag band DMAs into wbd[j%2] for capsule j.
        Prefetch DMAs round-robin over non-sync queues so they never block
        the latency-critical sync-queue DMAs (collective bounce, replicate)."""
        wbd = wbds[j % 2]
        wj = p_wj.tile([128, G, U], BF16, tag="wj", name=f"wj{j}")
        wj_tiles[j] = wj
        # W_j arranged [(d,khat), g, u]; one DMA per d (contiguous partitions).
        wj_v = wj.rearrange("(d k) g u -> d k g u", k=16)
        for d in range(D):
            src = bass.AP(
                w_dram, j * K * D * U + d * U,
                [[D * U, 16], [16 * D * U, G], [1, U]],
            )
            dma_engines[d % 2].dma_start(wj_v[d], src)
        # wbd[(khat,d), g, khat'*16+u] = W[j, g*16+khat, d, u] d(khat=khat')
        for r in range(16):
            src = bass.AP(
                w_dram, j * K * D * U + r * D * U,
                [[U, D], [16 * D * U, G], [1, U]],
            )
            dma_engines[r % 2].dma_start(
                wbd[8 * r:8 * r + 8, :, r * 16:(r + 1) * 16], src
            )

    def emit_uhat_mms(j, g_lo, g_hi, after=None):
        """PE matmuls + PSUM->SBUF copies for groups [g_lo, g_hi) of capsule j.
        `after`: ordering-only dep so the scheduler runs these in the
        AllReduce window (after the agree matmuls), not earlier."""
        if j not in A_tiles:
            A_tiles[j] = p_uhat.tile(
                [b_local, G, U, 16], BF16, tag="uhat", name=f"uhat{j}"
            )
        A = A_tiles[j]
        wbd = wbds[j % 2]
        last_mm = None
        for gq in range(g_lo // 2, g_hi // 2):
            ps = p_ps_uh.tile([b_local, 512], F32, tag="ps_uh", name="ps_uh")
            for i in range(2):
                g = gq * 2 + i
                last_mm = nc.tensor.matmul(
                    ps[:, i * 256:(i + 1) * 256],
                    x_a[:, g, :], wbd[:, g, :],
                    start=True, stop=True,
                )
                if after is not None:
                    add_dep_helper(last_mm.ins, after.ins, sync=False,
                                   reason="uhat after agree")
            # PSUM cols are (khat,u); store transposed as (u,khat) so the
            # agree matmuls stream contiguous khat runs.  Iterate (g,u,khat)
            # so the SBUF writes stay contiguous (strided PSUM reads).
            dst = A[:, gq * 2:gq * 2 + 2, :, :]
            src_v = ps.rearrange("b (g k u) -> b g k u", k=16, u=U)
            src_v = src_v.transpose((0, 1, 3, 2))
            if gq % 2 == 0:
                nc.vector.tensor_copy(dst, src_v)
            else:
                nc.scalar.copy(dst, src_v)
        return last_mm

    FILLER = 36  # warm-keeper matmuls riding the AllReduce window
    p_ps_f = es.enter_context(tc.tile_pool(name="ps_f", bufs=1, space="PSUM"))

    def emit_filler(j, n, after=None, sync_first=False):
        """Independent matmuls with no consumers: keep the PE HAM-warm
        during the AllReduce stall (results are discarded)."""
        wbd = wbds[j % 2]
        fps = p_ps_f.tile([b_local, 256], F32, tag="fps", name="fps")
        first_mm = last_mm = None
        for f in range(n):
            last_mm = nc.tensor.matmul(
                fps[:], x_a[:, f, :], wbd[:, f, :], start=True, stop=True,
            )
            if first_mm is None:
                first_mm = last_mm
            if after is not None:
                add_dep_helper(last_mm.ins, after.ins, sync=False,
                               reason="filler ordering")
        return first_mm, last_mm

    emit_w_loads(0)
    emit_uhat_mms(0, 0, G)
    emit_w_loads(1)
    pe_tail = None  # ordering anchor: last PE inst of the prev AR window

    for j in range(J):
        wj = wj_tiles.pop(j)
        A = A_tiles.pop(j)
        for t in range(ITERS):
            last = (j == J - 1) and (t == ITERS - 1)
            # softmax column j: c = e3[:,:,j] / den   (den kept incrementally)
            rec = p_small.tile([128, G], F32, tag="rec")
            c = p_small.tile([128, G], BF16, tag="c")
            nc.vector.reciprocal(rec[:], den[:])
            nc.vector.tensor_mul(c[:], e3[:, :, j], rec[:])
            # cW = W_j * c (c broadcast over u); two halves so the s matvec
            # can start while the second half is still computing
            cw = p_cw.tile([128, G, U], BF16, tag="cw")
            GH = G // 2
            for h in range(2):
                sl = slice(h * GH, (h + 1) * GH)
                nc.vector.tensor_mul(
                    cw[:, sl, :], wj[:, sl, :],
                    c[:, sl].unsqueeze(2).broadcast_to((128, GH, U)),
                )
            # s matvec: accumulate over groups
            s_ps = p_ps_s.tile([b_local, U], F32, tag="s_ps")
            for g in range(G):
                mm = nc.tensor.matmul(
                    s_ps[:], x_b[:, g, :], cw[:, g, :],
                    start=(g == 0), stop=(g == G - 1),
                )
                if g == 0 and pe_tail is not None:
                    add_dep_helper(mm.ins, pe_tail.ins, sync=False,
                                   reason="s after AR-window fillers")
            # squash: v = s * ssq / ((1+ssq)(sqrt(ssq)+EPS))
            # ssq via DVE fused mult+reduce (keeps ACT on the Sqrt table)
            s_sb = p_small.tile([b_local, U], F32, tag="s_sb")
            shadow = p_small.tile([b_local, U], F32, tag="shadow")
            ssq = p_small.tile([b_local, 1], F32, tag="ssq")
            sq1 = p_small.tile([b_local, 1], F32, tag="sq1")
            sqr = p_small.tile([b_local, 1], F32, tag="sqr")
            dn2 = p_small.tile([b_local, 1], F32, tag="dn2")
            rc2 = p_small.tile([b_local, 1], F32, tag="rc2")
            fac = p_small.tile([b_local, 1], F32, tag="fac")
            nc.scalar.activation(shadow[:], s_ps[:], AF.Square, accum_out=ssq[:])
            nc.scalar.sqrt(sqr[:], ssq[:])
            nc.vector.tensor_scalar_add(sq1[:], ssq[:], 1.0)
            nc.vector.scalar_tensor_tensor(
                dn2[:], sqr[:], EPS, sq1[:], ALU.add, ALU.mult
            )
            nc.vector.reciprocal(rc2[:], dn2[:])
            nc.vector.tensor_mul(fac[:], ssq[:], rc2[:])
            if not last:
                # preload the Exp ACT table during the AllReduce window
                # (anchored on fac so it runs after this squash)
                dxp = p_small.tile([b_local, 1], F32, tag="dxp")
                nc.scalar.activation(dxp[:], fac[:], AF.Exp)
            if t == ITERS - 1:
                v = p_v.tile([b_local, U], F32, tag="v")
                nc.vector.tensor_scalar_mul(v[:], s_ps[:], fac[:])
                nc.sync.dma_start(out_dram.ap()[:, j, :], v[:])
            if not last:
                # agree matvec: 3 thirds x 16 u accumulating matmuls (bf16)
                v_bf = p_v.tile([b_local, U], BF16, tag="v_bf")
                nc.vector.tensor_scalar_mul(v_bf[:], s_ps[:], fac[:])
                # one PSUM tile, thirds at bank-aligned offsets -> 1 copy out
                aps3 = p_ps_a.tile([1, 1536], F32, tag="ps_a3", name="ps_a3")
                agree_last = None
                for third in range(3):
                    rhs_base = A[:, third * 24:(third + 1) * 24, :, :]
                    for u in range(U):
                        agree_last = nc.tensor.matmul(
                            aps3[:, third * 512:third * 512 + 384],
                            v_bf[:, u:u + 1],
                            rhs_base[:, :, u, :],
                            start=(u == 0), stop=(u == U - 1),
                        )
            # overlap work for capsule j+1 during the AllReduce window
            fill_j = j + 1 if j + 1 < J else j
            if j + 1 < J:
                tail = emit_uhat_mms(j + 1, t * 24, (t + 1) * 24,
                                     after=None if last else agree_last)
                if t == 2 and j + 2 < J:
                    emit_w_loads(j + 2)
                _, tail = emit_filler(fill_j, FILLER, after=tail)
            else:
                tail = agree_last
            if last:
                pe_tail = None
                break
            cc_in = p_dram.tile([1, K], F32, tag="cc_in")
            cc_out = p_dram.tile([1, K], F32, tag="cc_out")
            agr_sb = p_agr.tile([1, K], F32, tag="agr_sb")
            nc.vector.tensor_copy(
                agr_sb.rearrange("p (a b) -> p a b", b=384),
                aps3.rearrange("p (a b) -> p a b", b=512)[:, :, 0:384],
            )
            nc.sync.dma_start(cc_in[:], agr_sb[:])
            cc_inst = nc.gpsimd.collective_compute(
                "AllReduce", ALU.add,
                replica_groups=replica_groups,
                ins=[cc_in[:].opt()], outs=[cc_out[:].opt()],
            )
            # second filler wave: starts the moment the collective
            # completes, keeping the PE warm through the post-AR
            # softmax/cW window until the next s matvec is ready
            f2_first, f2_last = emit_filler(fill_j, 40, after=tail)
            add_dep_helper(f2_first.ins, cc_inst.ins, sync=True,
                           reason="filler2 rides post-AR window")
            pe_tail = f2_last
            # e3[:,:,j] *= exp(agree): 2 DMAs fill partitions 0-31, then
            # 32-aligned DVE copies replicate to 128, then exp + multiply
            agr = p_agr.tile([128, G], F32, tag="agr")
            eag = p_agr.tile([128, G], F32, tag="eag")
            src = cc_out[0, :].rearrange("(g k) -> k g", k=16)
            agr_v = agr.rearrange("(d k) g -> d k g", k=16)
            nc.sync.dma_start(agr_v[0], src)
            nc.sync.dma_start(agr_v[1], src)
            for q in range(1, 4):
                nc.vector.tensor_copy(agr[32 * q:32 * (q + 1), :], agr[0:32, :])
            nc.scalar.activation(eag[:], agr[:], AF.Exp)
            # preload Sqrt table for the next squash (anchored on eag)
            dsq = p_small.tile([b_local, 1], F32, tag="dsq")
            nc.scalar.activation(dsq[:], eag[0:b_local, 0:1], AF.Sqrt)
            # delta = (eag-1)*e3_j keeps den incremental; then update e3
            delta = p_small.tile([128, G], F32, tag="delta")
            nc.vector.scalar_tensor_tensor(
                delta[:], eag[:], -1.0, e3[:, :, j], ALU.add, ALU.mult
            )
            nc.vector.tensor_mul(e3[:, :, j], e3[:, :, j], eag[:])
            nc.vector.tensor_add(den[:], den[:], delta[:])

    es.close()


def build_nc(n_cores=N_CORES, b_local=B // N_CORES):
    nc = bacc.Bacc(
        "TRN2", target_bir_lowering=False, debug=False,
        num_devices=n_cores,
    )
    x_dram = nc.dram_tensor("x_kd", [2, 128, G, b_local], BF16, kind="ExternalInput")
    w_dram = nc.dram_tensor("w", [J, K, D, U], BF16, kind="ExternalInput")
    out_dram = nc.dram_tensor("out", [b_local, J, U], F32, kind="ExternalOutput")
    rg = [list(range(n_cores))]
    with tile.TileContext(nc) as tc:
        capsnet_body(tc, nc, x_dram, w_dram, out_dram, rg, b_local)
    nc.compile()
    return nc


def shard_x(x_full):
    """x_full [B,1152,8,1] -> per-core [2, 128, G, b] bf16: x_a (p=khat*8+d)
    and x_b (p=d*16+khat) stacked."""
    b_local = x_full.shape[0] // N_CORES
    shards = []
    for i in range(N_CORES):
        xs = np.ascontiguousarray(
            x_full[i * b_local:(i + 1) * b_local, :, :, 0], dtype=np.float32
        )
        r = xs.reshape(b_local, G, 16, D)
        x_a = r.transpose(2, 3, 1, 0).reshape(128, G, b_local)  # khat-major
        x_b = r.transpose(3, 2, 1, 0).reshape(128, G, b_local)  # d-major
        shards.append(np.ascontiguousarray(
            np.stack([x_a, x_b]).astype(ml_dtypes.bfloat16)))
    return shards


_NC_CACHE = {}


def kernel(inputs, W, num_outputs):
    assert int(num_outputs) == J
    x_full = np.asarray(inputs, dtype=np.float32)
    w = np.ascontiguousarray(
        np.asarray(W, dtype=np.float32).astype(ml_dtypes.bfloat16))
    assert x_full.shape == (B, K, D, 1) and w.shape == (J, K, D, U)

    if "nc" not in _NC_CACHE:
        _NC_CACHE["nc"] = build_nc()
    nc = _NC_CACHE["nc"]

    shards = shard_x(x_full)
    in_maps = [{"x_kd": shards[i], "w": w} for i in range(N_CORES)]
    res = bass_utils.run_bass_kernel_spmd(
        nc, in_maps, core_ids=list(range(N_CORES))
    )
    outs = [res.results[i]["out"] for i in range(N_CORES)]  # [b, J, U] each
    full = np.concatenate(outs, axis=0)  # [B, J, U]
    return full[..., None].astype(np.float32)


# revision 62
# speedup vs baseline: 1.0055x; 1.0055x over previous
"""CapsNet dynamic-routing kernel for Trainium2 (8 NeuronCores, SPMD).

Math (see reference):
  u_hat[j,b,k,u] = sum_d W[j,k,d,u] * x[b,k,d]
  for j in 0..9:  (sequential, b_IJ carried)
    3 routing iterations:
      c_k      = softmax(b_IJ, axis=1)[:, j]
      s[b,u]   = sum_k c_k u_hat[j,b,k,u]
      v        = squash(s)
      agree[k] = sum_{b,u} u_hat[j,b,k,u] v[b,u]   (sum over FULL batch)
      b_IJ[:, j] += agree
  out[b,j,u] = v (last iteration of each j)

Distribution: data-parallel over batch (64 per core).  The only cross-core
quantity is agree -> AllReduce[1152] each routing iteration (29 total; the
final (j=9,t=2) update is dead and skipped).

All matmul operands are bf16 (fp32 matmuls double-pass on the PE,
fp32_mode=LOW_HIGH); accumulation is fp32 in PSUM, and all routing state
(e3/softmax/squash/AllReduce) stays fp32.

Per-core layouts (G=72 groups of 16 k).  DMA partition ranges must be
contiguous, which forces two partition orders:
  k-major  p = khat*8 + d   : wbd (block-diag W_j band writes), x_a
  d-major  p = d*16 + khat  : wj, cW, c, e3, agr (replicate writes), x_b
  x_a/x_b SBUF [128, G, 64] bf16     : x[b, g*16+khat, d] in each order
  wbd     SBUF [128, G, 256] bf16    : block-diag W_j; col c=(khat',u)
  u_hat A SBUF [64=b, G, 16=khat, 16=u] bf16  (from PE matmuls via PSUM)
  e3      SBUF [128, G, 10] fp32     : exp(b_IJ); softmax needs no
                                       max-subtraction (|b_IJ| < 5 here)
s-matvec avoids u_hat entirely:  s = sum_g x_b[:,g,:]^T @ (c*W_j)[:,g,:]
agree: 48 accumulating matmuls  lhsT=v_bf[:,u], rhs=A[:, g-third, :, u].
u_hat for capsule j+1 is emitted interleaved into j's routing iterations
so the PE has work during the AllReduce stalls (HAM stays warm).
"""

import numpy as np
import ml_dtypes

import concourse.bass as bass
import concourse.bacc as bacc
import concourse.mybir as mybir
import concourse.tile as tile
from concourse.tile import add_dep_helper
from concourse import bass_utils

F32 = mybir.dt.float32
BF16 = mybir.dt.bfloat16
AF = mybir.ActivationFunctionType
ALU = mybir.AluOpType

J = 10        # output capsules
K = 1152      # input capsules
D = 8         # in dim
U = 16        # out dim
B = 512       # batch
N_CORES = 8
ITERS = 3
EPS = 1e-7
G = K // 16   # 72 groups of 16 k


def capsnet_body(tc, nc, x_dram, w_dram, out_dram, replica_groups, b_local):
    """Emit the per-core program. x [2,128,G,b] bf16, w [J,K,D,U] bf16,
    out [b, J, U] fp32."""
    from contextlib import ExitStack
    es = ExitStack()
    p_const = es.enter_context(tc.tile_pool(name="const", bufs=1))
    p_uhat = es.enter_context(tc.tile_pool(name="uhat", bufs=2))
    p_wj = es.enter_context(tc.tile_pool(name="wj", bufs=2))
    p_cw = es.enter_context(tc.tile_pool(name="cw", bufs=2))
    p_small = es.enter_context(tc.tile_pool(name="small", bufs=4))
    p_v = es.enter_context(tc.tile_pool(name="vpool", bufs=2))
    p_agr = es.enter_context(tc.tile_pool(name="agr", bufs=2))
    p_ps_uh = es.enter_context(tc.tile_pool(name="ps_uh", bufs=2, space="PSUM"))
    p_ps_s = es.enter_context(tc.tile_pool(name="ps_s", bufs=2, space="PSUM"))
    p_ps_a = es.enter_context(tc.tile_pool(name="ps_a", bufs=1, space="PSUM"))
    p_dram = es.enter_context(tc.tile_pool(name="dram", bufs=4, space="DRAM"))

    # ---- persistent tiles
    x_a = p_const.tile([128, G, b_local], BF16, tag="x_a")
    x_b = p_const.tile([128, G, b_local], BF16, tag="x_b")
    wbd_a = p_const.tile([128, G, 256], BF16, tag="wbd_a")
    wbd_b = p_const.tile([128, G, 256], BF16, tag="wbd_b")
    e3 = p_const.tile([128, G, J], F32, tag="e3")
    den = p_const.tile([128, G], F32, tag="den")  # sum_j e3, kept incrementally
    wbds = [wbd_a, wbd_b]

    nc.gpsimd.dma_start(x_a[:], x_dram.ap()[0])
    nc.gpsimd.dma_start(x_b[:], x_dram.ap()[1])
    nc.vector.memset(wbd_a[:], 0.0)
    nc.vector.memset(wbd_b[:], 0.0)
    nc.vector.memset(e3[:], 1.0)
    nc.vector.memset(den[:], float(J))

    wj_tiles = {}
    A_tiles = {}
    dma_engines = [nc.gpsimd, nc.scalar]

    def emit_w_loads(j):
        """wj DMA + block-diag band DMAs into wbd[j%2] for capsule j.
        Prefetch DMAs round-robin over non-sync queues so they never block
        the latency-critical sync-queue DMAs (collective bounce, replicate)."""
        wbd = wbds[j % 2]
        wj = p_wj.tile([128, G, U], BF16, tag="wj", name=f"wj{j}")
        wj_tiles[j] = wj
        # W_j arranged [(d,khat), g, u]; one DMA per d (contiguous partitions).
        wj_v = wj.rearrange("(d k) g u -> d k g u", k=16)
        for d in range(D):
            src = bass.AP(
                w_dram, j * K * D * U + d * U,
                [[D * U, 16], [16 * D * U, G], [1, U]],
            )
            dma_engines[d % 2].dma_start(wj_v[d], src)
        # wbd[(khat,d), g, khat'*16+u] = W[j, g*16+khat, d, u] d(khat=khat')
        for r in range(16):
            src = bass.AP(
                w_dram, j * K * D * U + r * D * U,
                [[U, D], [16 * D * U, G], [1, U]],
            )
            dma_engines[r % 2].dma_start(
                wbd[8 * r:8 * r + 8, :, r * 16:(r + 1) * 16], src
            )

    def emit_uhat_mms(j, g_lo, g_hi, after=None):
        """PE matmuls + PSUM->SBUF copies for groups [g_lo, g_hi) of capsule j.
        `after`: ordering-only dep so the scheduler runs these in the
        AllReduce window (after the agree matmuls), not earlier."""
        if j not in A_tiles:
            A_tiles[j] = p_uhat.tile(
                [b_local, G, U, 16], BF16, tag="uhat", name=f"uhat{j}"
            )
        A = A_tiles[j]
        wbd = wbds[j % 2]
        last_mm = None
        for gq in range(g_lo // 2, g_hi // 2):
            ps = p_ps_uh.tile([b_local, 512], F32, tag="ps_uh", name="ps_uh")
            for i in range(2):
                g = gq * 2 + i
                last_mm = nc.tensor.matmul(
                    ps[:, i * 256:(i + 1) * 256],
                    x_a[:, g, :], wbd[:, g, :],
                    start=True, stop=True,
                )
                if after is not None:
                    add_dep_helper(last_mm.ins, after.ins, sync=False,
                                   reason="uhat after agree")
            # PSUM cols are (khat,u); store transposed as (u,khat) so the
            # agree matmuls stream contiguous khat runs.  Iterate (g,u,khat)
            # so the SBUF writes stay contiguous (strided PSUM reads).
            dst = A[:, gq * 2:gq * 2 + 2, :, :]
            src_v = ps.rearrange("b (g k u) -> b g k u", k=16, u=U)
            src_v = src_v.transpose((0, 1, 3, 2))
            if gq % 2 == 0:
                nc.vector.tensor_copy(dst, src_v)
            else:
                nc.scalar.copy(dst, src_v)
        return last_mm

    FILLER = 36  # warm-keeper matmuls riding the AllReduce window
    p_ps_f = es.enter_context(tc.tile_pool(name="ps_f", bufs=1, space="PSUM"))

    def emit_filler(j, n, after=None, sync_first=False):
        """Independent matmuls with no consumers: keep the PE HAM-warm
        during the AllReduce stall (results are discarded)."""
        wbd = wbds[j % 2]
        fps = p_ps_f.tile([b_local, 256], F32, tag="fps", name="fps")
        first_mm = last_mm = None
        for f in range(n):
            last_mm = nc.tensor.matmul(
                fps[:], x_a[:, f, :], wbd[:, f, :], start=True, stop=True,
            )
            if first_mm is None:
                first_mm = last_mm
            if after is not None:
                add_dep_helper(last_mm.ins, after.ins, sync=False,
                               reason="filler ordering")
        return first_mm, last_mm

    emit_w_loads(0)
    emit_uhat_mms(0, 0, G)
    emit_w_loads(1)
    pe_tail = None  # ordering anchor: last PE inst of the prev AR window

    for j in range(J):
        wj = wj_tiles.pop(j)
        A = A_tiles.pop(j)
        for t in range(ITERS):
            last = (j == J - 1) and (t == ITERS - 1)
            # softmax column j: c = e3[:,:,j] / den   (den kept incrementally)
            rec = p_small.tile([128, G], F32, tag="rec")
            c = p_small.tile([128, G], BF16, tag="c")
            nc.vector.reciprocal(rec[:], den[:])
            nc.vector.tensor_mul(c[:], e3[:, :, j], rec[:])
            # cW = W_j * c (c broadcast over u); two halves so the s matvec
            # can start while the second half is still computing
            cw = p_cw.tile([128, G, U], BF16, tag="cw")
            GH = G // 2
            for h in range(2):
                sl = slice(h * GH, (h + 1) * GH)
                nc.vector.tensor_mul(
                    cw[:, sl, :], wj[:, sl, :],
                    c[:, sl].unsqueeze(2).broadcast_to((128, GH, U)),
                )
            # s matvec: accumulate over groups
            s_ps = p_ps_s.tile([b_local, U], F32, tag="s_ps")
            for g in range(G):
                mm = nc.tensor.matmul(
                    s_ps[:], x_b[:, g, :], cw[:, g, :],
                    start=(g == 0), stop=(g == G - 1),
                )
                if g == 0 and pe_tail is not None:
                    add_dep_helper(mm.ins, pe_tail.ins, sync=False,
                                   reason="s after AR-window fillers")
            # squash: v = s * ssq / ((1+ssq)(sqrt(ssq)+EPS))
            # ssq via DVE fused mult+reduce (keeps ACT on the Sqrt table)
            s_sb = p_small.tile([b_local, U], F32, tag="s_sb")
            shadow = p_small.tile([b_local, U], F32, tag="shadow")
            ssq = p_small.tile([b_local, 1], F32, tag="ssq")
            sq1 = p_small.tile([b_local, 1], F32, tag="sq1")
            sqr = p_small.tile([b_local, 1], F32, tag="sqr")
            dn2 = p_small.tile([b_local, 1], F32, tag="dn2")
            rc2 = p_small.tile([b_local, 1], F32, tag="rc2")
            fac = p_small.tile([b_local, 1], F32, tag="fac")
            nc.vector.tensor_copy(s_sb[:], s_ps[:])
            nc.vector.tensor_mul(shadow[:], s_sb[:], s_sb[:])
            nc.vector.tensor_reduce(ssq[:], shadow[:], mybir.AxisListType.X, ALU.add)
            nc.scalar.sqrt(sqr[:], ssq[:])
            nc.vector.tensor_scalar_add(sq1[:], ssq[:], 1.0)
            nc.vector.scalar_tensor_tensor(
                dn2[:], sqr[:], EPS, sq1[:], ALU.add, ALU.mult
            )
            nc.vector.reciprocal(rc2[:], dn2[:])
            nc.vector.tensor_mul(fac[:], ssq[:], rc2[:])
            if not last:
                # preload the Exp ACT table during the AllReduce window
                # (anchored on fac so it runs after this squash)
                dxp = p_small.tile([b_local, 1], F32, tag="dxp")
                nc.scalar.activation(dxp[:], fac[:], AF.Exp)
            if t == ITERS - 1:
                v = p_v.tile([b_local, U], F32, tag="v")
                nc.vector.tensor_scalar_mul(v[:], s_ps[:], fac[:])
                nc.sync.dma_start(out_dram.ap()[:, j, :], v[:])
            if not last:
                # agree matvec: 3 thirds x 16 u accumulating matmuls (bf16)
                v_bf = p_v.tile([b_local, U], BF16, tag="v_bf")
                nc.vector.tensor_scalar_mul(v_bf[:], s_ps[:], fac[:])
                # one PSUM tile, thirds at bank-aligned offsets -> 1 copy out.
                # u outer / third inner: consecutive matmuls hit different
                # PSUM banks, so accumulate drains overlap the next fill.
                aps3 = p_ps_a.tile([1, 1536], F32, tag="ps_a3", name="ps_a3")
                agree_last = None
                for u in range(U):
                    for third in range(3):
                        rhs_base = A[:, third * 24:(third + 1) * 24, :, :]
                        agree_last = nc.tensor.matmul(
                            aps3[:, third * 512:third * 512 + 384],
                            v_bf[:, u:u + 1],
                            rhs_base[:, :, u, :],
                            start=(u == 0), stop=(u == U - 1),
                        )
            # overlap work for capsule j+1 during the AllReduce window
            fill_j = j + 1 if j + 1 < J else j
            if j + 1 < J:
                tail = emit_uhat_mms(j + 1, t * 24, (t + 1) * 24,
                                     after=None if last else agree_last)
                if t == 2 and j + 2 < J:
                    emit_w_loads(j + 2)
                _, tail = emit_filler(fill_j, FILLER, after=tail)
            else:
                tail = agree_last
            if last:
                pe_tail = None
                break
            cc_in = p_dram.tile([1, K], F32, tag="cc_in")
            cc_out = p_dram.tile([1, K], F32, tag="cc_out")
            agr_sb = p_agr.tile([1, K], F32, tag="agr_sb")
            nc.vector.tensor_copy(
                agr_sb.rearrange("p (a b) -> p a b", b=384),
                aps3.rearrange("p (a b) -> p a b", b=512)[:, :, 0:384],
            )
            nc.sync.dma_start(cc_in[:], agr_sb[:])
            cc_inst = nc.gpsimd.collective_compute(
                "AllReduce", ALU.add,
                replica_groups=replica_groups,
                ins=[cc_in[:].opt()], outs=[cc_out[:].opt()],
            )
            # second filler wave: starts the moment the collective
            # completes, keeping the PE warm through the post-AR
            # softmax/cW window until the next s matvec is ready
            f2_first, f2_last = emit_filler(fill_j, 40, after=tail)
            add_dep_helper(f2_first.ins, cc_inst.ins, sync=True,
                           reason="filler2 rides post-AR window")
            pe_tail = f2_last
            # e3[:,:,j] *= exp(agree): 2 DMAs fill partitions 0-31, then
            # 32-aligned DVE copies replicate to 128, then exp + multiply
            agr = p_agr.tile([128, G], F32, tag="agr")
            eag = p_agr.tile([128, G], F32, tag="eag")
            src = cc_out[0, :].rearrange("(g k) -> k g", k=16)
            agr_v = agr.rearrange("(d k) g -> d k g", k=16)
            nc.sync.dma_start(agr_v[0], src)
            nc.sync.dma_start(agr_v[1], src)
            for q in range(1, 4):
                nc.vector.tensor_copy(agr[32 * q:32 * (q + 1), :], agr[0:32, :])
            nc.scalar.activation(eag[:], agr[:], AF.Exp)
            # preload Sqrt table for the next squash (anchored on eag)
            dsq = p_small.tile([b_local, 1], F32, tag="dsq")
            nc.scalar.activation(dsq[:], eag[0:b_local, 0:1], AF.Sqrt)
            # delta = (eag-1)*e3_j keeps den incremental; then update e3
            delta = p_small.tile([128, G], F32, tag="delta")
            nc.vector.scalar_tensor_tensor(
                delta[:], eag[:], -1.0, e3[:, :, j], ALU.add, ALU.mult
            )
            nc.vector.tensor_mul(e3[:, :, j], e3[:, :, j], eag[:])
            nc.vector.tensor_add(den[:], den[:], delta[:])

    es.close()


def build_nc(n_cores=N_CORES, b_local=B // N_CORES):
    nc = bacc.Bacc(
        "TRN2", target_bir_lowering=False, debug=False,
        num_devices=n_cores,
    )
    x_dram = nc.dram_tensor("x_kd", [2, 128, G, b_local], BF16, kind="ExternalInput")
    w_dram = nc.dram_tensor("w", [J, K, D, U], BF16, kind="ExternalInput")
    out_dram = nc.dram_tensor("out", [b_local, J, U], F32, kind="ExternalOutput")
    rg = [list(range(n_cores))]
    with tile.TileContext(nc) as tc:
        capsnet_body(tc, nc, x_dram, w_dram, out_dram, rg, b_local)
    nc.compile()
    return nc


def shard_x(x_full):
    """x_full [B,1152,8,1] -> per-core [2, 128, G, b] bf16: x_a (p=khat*8+d)
    and x_b (p=d*16+khat) stacked."""
    b_local = x_full.shape[0] // N_CORES
    shards = []
    for i in range(N_CORES):
        xs = np.ascontiguousarray(
            x_full[i * b_local:(i + 1) * b_local, :, :, 0], dtype=np.float32
        )
        r = xs.reshape(b_local, G, 16, D)
        x_a = r.transpose(2, 3, 1, 0).reshape(128, G, b_local)  # khat-major
        x_b = r.transpose(3, 2, 1, 0).reshape(128, G, b_local)  # d-major
        shards.append(np.ascontiguousarray(
            np.stack([x_a, x_b]).astype(ml_dtypes.bfloat16)))
    return shards


_NC_CACHE = {}


def kernel(inputs, W, num_outputs):
    assert int(num_outputs) == J
    x_full = np.asarray(inputs, dtype=np.float32)
    w = np.ascontiguousarray(
        np.asarray(W, dtype=np.float32).astype(ml_dtypes.bfloat16))
    assert x_full.shape == (B, K, D, 1) and w.shape == (J, K, D, U)

    if "nc" not in _NC_CACHE:
        _NC_CACHE["nc"] = build_nc()
    nc = _NC_CACHE["nc"]

    shards = shard_x(x_full)
    in_maps = [{"x_kd": shards[i], "w": w} for i in range(N_CORES)]
    res = bass_utils.run_bass_kernel_spmd(
        nc, in_maps, core_ids=list(range(N_CORES))
    )
    outs = [res.results[i]["out"] for i in range(N_CORES)]  # [b, J, U] each
    full = np.concatenate(outs, axis=0)  # [B, J, U]
    return full[..., None].astype(np.float32)


# revision 64
# speedup vs baseline: 1.0350x; 1.0294x over previous
"""CapsNet dynamic-routing kernel for Trainium2 (8 NeuronCores, SPMD).

Math (see reference):
  u_hat[j,b,k,u] = sum_d W[j,k,d,u] * x[b,k,d]
  for j in 0..9:  (sequential, b_IJ carried)
    3 routing iterations:
      c_k      = softmax(b_IJ, axis=1)[:, j]
      s[b,u]   = sum_k c_k u_hat[j,b,k,u]
      v        = squash(s)
      agree[k] = sum_{b,u} u_hat[j,b,k,u] v[b,u]   (sum over FULL batch)
      b_IJ[:, j] += agree
  out[b,j,u] = v (last iteration of each j)

Distribution: data-parallel over batch (64 per core).  The only cross-core
quantity is agree -> AllReduce[1152] each routing iteration (29 total; the
final (j=9,t=2) update is dead and skipped).

All matmul operands are bf16 (fp32 matmuls double-pass on the PE,
fp32_mode=LOW_HIGH); accumulation is fp32 in PSUM, and all routing state
(e3/softmax/squash/AllReduce) stays fp32.

Per-core layouts (G=72 groups of 16 k).  DMA partition ranges must be
contiguous, which forces two partition orders:
  k-major  p = khat*8 + d   : wbd (block-diag W_j band writes), x_a
  d-major  p = d*16 + khat  : wj, cW, c, e3, agr (replicate writes), x_b
  x_a/x_b SBUF [128, G, 64] bf16     : x[b, g*16+khat, d] in each order
  wbd     SBUF [128, G, 256] bf16    : block-diag W_j; col c=(khat',u)
  u_hat A SBUF [64=b, G, 16=khat, 16=u] bf16  (from PE matmuls via PSUM)
  e3      SBUF [128, G, 10] fp32     : exp(b_IJ); softmax needs no
                                       max-subtraction (|b_IJ| < 5 here)
s-matvec avoids u_hat entirely:  s = sum_g x_b[:,g,:]^T @ (c*W_j)[:,g,:]
agree: 48 accumulating matmuls  lhsT=v_bf[:,u], rhs=A[:, g-third, :, u].
u_hat for capsule j+1 is emitted interleaved into j's routing iterations
so the PE has work during the AllReduce stalls (HAM stays warm).
"""

import numpy as np
import ml_dtypes

import concourse.bass as bass
import concourse.bacc as bacc
import concourse.mybir as mybir
import concourse.tile as tile
from concourse.tile import add_dep_helper
from concourse import bass_utils

F32 = mybir.dt.float32
BF16 = mybir.dt.bfloat16
AF = mybir.ActivationFunctionType
ALU = mybir.AluOpType

J = 10        # output capsules
K = 1152      # input capsules
D = 8         # in dim
U = 16        # out dim
B = 512       # batch
N_CORES = 8
ITERS = 3
EPS = 1e-7
G = K // 16   # 72 groups of 16 k


def capsnet_body(tc, nc, x_dram, w_dram, out_dram, replica_groups, b_local):
    """Emit the per-core program. x [2,128,G,b] bf16, w [J,K,D,U] bf16,
    out [b, J, U] fp32."""
    from contextlib import ExitStack
    es = ExitStack()
    p_const = es.enter_context(tc.tile_pool(name="const", bufs=1))
    p_uhat = es.enter_context(tc.tile_pool(name="uhat", bufs=2))
    p_wj = es.enter_context(tc.tile_pool(name="wj", bufs=2))
    p_cw = es.enter_context(tc.tile_pool(name="cw", bufs=2))
    p_small = es.enter_context(tc.tile_pool(name="small", bufs=4))
    p_v = es.enter_context(tc.tile_pool(name="vpool", bufs=2))
    p_agr = es.enter_context(tc.tile_pool(name="agr", bufs=2))
    p_ps_uh = es.enter_context(tc.tile_pool(name="ps_uh", bufs=2, space="PSUM"))
    p_ps_s = es.enter_context(tc.tile_pool(name="ps_s", bufs=2, space="PSUM"))
    p_ps_a = es.enter_context(tc.tile_pool(name="ps_a", bufs=1, space="PSUM"))
    p_dram = es.enter_context(tc.tile_pool(name="dram", bufs=4, space="DRAM"))

    # ---- persistent tiles
    x_a = p_const.tile([128, G, b_local], BF16, tag="x_a")
    x_b = p_const.tile([128, G, b_local], BF16, tag="x_b")
    wbd_a = p_const.tile([128, G, 256], BF16, tag="wbd_a")
    wbd_b = p_const.tile([128, G, 256], BF16, tag="wbd_b")
    e3 = p_const.tile([128, G, J], F32, tag="e3")
    den = p_const.tile([128, G], F32, tag="den")  # sum_j e3, kept incrementally
    wbds = [wbd_a, wbd_b]

    nc.gpsimd.dma_start(x_a[:], x_dram.ap()[0])
    nc.gpsimd.dma_start(x_b[:], x_dram.ap()[1])
    nc.vector.memset(wbd_a[:], 0.0)
    nc.vector.memset(wbd_b[:], 0.0)
    nc.vector.memset(e3[:], 1.0)
    nc.vector.memset(den[:], float(J))

    wj_tiles = {}
    A_tiles = {}
    dma_engines = [nc.gpsimd, nc.scalar]

    def emit_w_loads(j):
        """wj DMA + block-diag band DMAs into wbd[j%2] for capsule j.
        Prefetch DMAs round-robin over non-sync queues so they never block
        the latency-critical sync-queue DMAs (collective bounce, replicate)."""
        wbd = wbds[j % 2]
        wj = p_wj.tile([128, G, U], BF16, tag="wj", name=f"wj{j}")
        wj_tiles[j] = wj
        # W_j arranged [(d,khat), g, u]; one DMA per d (contiguous partitions).
        wj_v = wj.rearrange("(d k) g u -> d k g u", k=16)
        for d in range(D):
            src = bass.AP(
                w_dram, j * K * D * U + d * U,
                [[D * U, 16], [16 * D * U, G], [1, U]],
            )
            dma_engines[d % 2].dma_start(wj_v[d], src)
        # wbd[(khat,d), g, khat'*16+u] = W[j, g*16+khat, d, u] d(khat=khat')
        for r in range(16):
            src = bass.AP(
                w_dram, j * K * D * U + r * D * U,
                [[U, D], [16 * D * U, G], [1, U]],
            )
            dma_engines[r % 2].dma_start(
                wbd[8 * r:8 * r + 8, :, r * 16:(r + 1) * 16], src
            )

    def emit_uhat_mms(j, g_lo, g_hi, after=None):
        """PE matmuls + PSUM->SBUF copies for groups [g_lo, g_hi) of capsule j.
        `after`: ordering-only dep so the scheduler runs these in the
        AllReduce window (after the agree matmuls), not earlier."""
        if j not in A_tiles:
            A_tiles[j] = p_uhat.tile(
                [b_local, G, U, 16], BF16, tag="uhat", name=f"uhat{j}"
            )
        A = A_tiles[j]
        wbd = wbds[j % 2]
        last_mm = None
        for gq in range(g_lo // 2, g_hi // 2):
            ps = p_ps_uh.tile([b_local, 512], F32, tag="ps_uh", name="ps_uh")
            for i in range(2):
                g = gq * 2 + i
                last_mm = nc.tensor.matmul(
                    ps[:, i * 256:(i + 1) * 256],
                    x_a[:, g, :], wbd[:, g, :],
                    start=True, stop=True,
                )
                if after is not None:
                    add_dep_helper(last_mm.ins, after.ins, sync=False,
                                   reason="uhat after agree")
            # PSUM cols are (khat,u); store transposed as (u,khat) so the
            # agree matmuls stream contiguous khat runs.  Iterate (g,u,khat)
            # so the SBUF writes stay contiguous (strided PSUM reads).
            # all copies on DVE: a Copy activation on ACT would evict the
            # preloaded Sqrt/Exp tables and put a reload on the critical path
            dst = A[:, gq * 2:gq * 2 + 2, :, :]
            src_v = ps.rearrange("b (g k u) -> b g k u", k=16, u=U)
            src_v = src_v.transpose((0, 1, 3, 2))
            nc.vector.tensor_copy(dst, src_v)
        return last_mm

    FILLER = 36  # warm-keeper matmuls riding the AllReduce window
    p_ps_f = es.enter_context(tc.tile_pool(name="ps_f", bufs=1, space="PSUM"))

    def emit_filler(j, n, after=None, sync_first=False):
        """Independent matmuls with no consumers: keep the PE HAM-warm
        during the AllReduce stall (results are discarded)."""
        wbd = wbds[j % 2]
        fps = p_ps_f.tile([b_local, 256], F32, tag="fps", name="fps")
        first_mm = last_mm = None
        for f in range(n):
            last_mm = nc.tensor.matmul(
                fps[:], x_a[:, f, :], wbd[:, f, :], start=True, stop=True,
            )
            if first_mm is None:
                first_mm = last_mm
            if after is not None:
                add_dep_helper(last_mm.ins, after.ins, sync=False,
                               reason="filler ordering")
        return first_mm, last_mm

    emit_w_loads(0)
    emit_uhat_mms(0, 0, G)
    emit_w_loads(1)
    pe_tail = None  # ordering anchor: last PE inst of the prev AR window

    for j in range(J):
        wj = wj_tiles.pop(j)
        A = A_tiles.pop(j)
        for t in range(ITERS):
            last = (j == J - 1) and (t == ITERS - 1)
            # softmax column j: c = e3[:,:,j] / den   (den kept incrementally)
            rec = p_small.tile([128, G], F32, tag="rec")
            c = p_small.tile([128, G], BF16, tag="c")
            nc.vector.reciprocal(rec[:], den[:])
            nc.vector.tensor_mul(c[:], e3[:, :, j], rec[:])
            # cW = W_j * c (c broadcast over u); two halves so the s matvec
            # can start while the second half is still computing
            cw = p_cw.tile([128, G, U], BF16, tag="cw")
            GH = G // 2
            for h in range(2):
                sl = slice(h * GH, (h + 1) * GH)
                nc.vector.tensor_mul(
                    cw[:, sl, :], wj[:, sl, :],
                    c[:, sl].unsqueeze(2).broadcast_to((128, GH, U)),
                )
            # s matvec: accumulate over groups
            s_ps = p_ps_s.tile([b_local, U], F32, tag="s_ps")
            for g in range(G):
                mm = nc.tensor.matmul(
                    s_ps[:], x_b[:, g, :], cw[:, g, :],
                    start=(g == 0), stop=(g == G - 1),
                )
                if g == 0 and pe_tail is not None:
                    add_dep_helper(mm.ins, pe_tail.ins, sync=False,
                                   reason="s after AR-window fillers")
            # squash: v = s * ssq / ((1+ssq)(sqrt(ssq)+EPS))
            # ssq via DVE fused mult+reduce (keeps ACT on the Sqrt table)
            s_sb = p_small.tile([b_local, U], F32, tag="s_sb")
            shadow = p_small.tile([b_local, U], F32, tag="shadow")
            ssq = p_small.tile([b_local, 1], F32, tag="ssq")
            sq1 = p_small.tile([b_local, 1], F32, tag="sq1")
            sqr = p_small.tile([b_local, 1], F32, tag="sqr")
            dn2 = p_small.tile([b_local, 1], F32, tag="dn2")
            rc2 = p_small.tile([b_local, 1], F32, tag="rc2")
            fac = p_small.tile([b_local, 1], F32, tag="fac")
            nc.vector.tensor_copy(s_sb[:], s_ps[:])
            nc.vector.tensor_mul(shadow[:], s_sb[:], s_sb[:])
            nc.vector.tensor_reduce(ssq[:], shadow[:], mybir.AxisListType.X, ALU.add)
            nc.scalar.sqrt(sqr[:], ssq[:])
            nc.vector.tensor_scalar_add(sq1[:], ssq[:], 1.0)
            nc.vector.scalar_tensor_tensor(
                dn2[:], sqr[:], EPS, sq1[:], ALU.add, ALU.mult
            )
            nc.vector.reciprocal(rc2[:], dn2[:])
            nc.vector.tensor_mul(fac[:], ssq[:], rc2[:])
            if not last:
                # preload the Exp ACT table during the AllReduce window
                # (anchored on fac so it runs after this squash)
                dxp = p_small.tile([b_local, 1], F32, tag="dxp")
                nc.scalar.activation(dxp[:], fac[:], AF.Exp)
            if t == ITERS - 1:
                v = p_v.tile([b_local, U], F32, tag="v")
                nc.vector.tensor_scalar_mul(v[:], s_ps[:], fac[:])
                nc.sync.dma_start(out_dram.ap()[:, j, :], v[:])
            if not last:
                # agree matvec: 3 thirds x 16 u accumulating matmuls (bf16)
                v_bf = p_v.tile([b_local, U], BF16, tag="v_bf")
                nc.vector.tensor_scalar_mul(v_bf[:], s_ps[:], fac[:])
                # one PSUM tile, thirds at bank-aligned offsets -> 1 copy out.
                # u outer / third inner: consecutive matmuls hit different
                # PSUM banks, so accumulate drains overlap the next fill.
                aps3 = p_ps_a.tile([1, 1536], F32, tag="ps_a3", name="ps_a3")
                agree_last = None
                for u in range(U):
                    for third in range(3):
                        rhs_base = A[:, third * 24:(third + 1) * 24, :, :]
                        agree_last = nc.tensor.matmul(
                            aps3[:, third * 512:third * 512 + 384],
                            v_bf[:, u:u + 1],
                            rhs_base[:, :, u, :],
                            start=(u == 0), stop=(u == U - 1),
                        )
            # overlap work for capsule j+1 during the AllReduce window
            fill_j = j + 1 if j + 1 < J else j
            if j + 1 < J:
                tail = emit_uhat_mms(j + 1, t * 24, (t + 1) * 24,
                                     after=None if last else agree_last)
                if t == 2 and j + 2 < J:
                    emit_w_loads(j + 2)
                _, tail = emit_filler(fill_j, FILLER, after=tail)
            else:
                tail = agree_last
            if last:
                pe_tail = None
                break
            cc_in = p_dram.tile([1, K], F32, tag="cc_in")
            cc_out = p_dram.tile([1, K], F32, tag="cc_out")
            agr_sb = p_agr.tile([1, K], F32, tag="agr_sb")
            nc.vector.tensor_copy(
                agr_sb.rearrange("p (a b) -> p a b", b=384),
                aps3.rearrange("p (a b) -> p a b", b=512)[:, :, 0:384],
            )
            nc.sync.dma_start(cc_in[:], agr_sb[:])
            cc_inst = nc.gpsimd.collective_compute(
                "AllReduce", ALU.add,
                replica_groups=replica_groups,
                ins=[cc_in[:].opt()], outs=[cc_out[:].opt()],
            )
            # second filler wave: starts the moment the collective
            # completes, keeping the PE warm through the post-AR
            # softmax/cW window until the next s matvec is ready
            f2_first, f2_last = emit_filler(fill_j, 60, after=tail)
            add_dep_helper(f2_first.ins, cc_inst.ins, sync=True,
                           reason="filler2 rides post-AR window")
            pe_tail = f2_last
            # e3[:,:,j] *= exp(agree): 2 DMAs fill partitions 0-31, then
            # 32-aligned DVE copies replicate to 128, then exp + multiply
            agr = p_agr.tile([128, G], F32, tag="agr")
            eag = p_agr.tile([128, G], F32, tag="eag")
            src = cc_out[0, :].rearrange("(g k) -> k g", k=16)
            agr_v = agr.rearrange("(d k) g -> d k g", k=16)
            nc.sync.dma_start(agr_v[0], src)
            nc.sync.dma_start(agr_v[1], src)
            for q in range(1, 4):
                nc.vector.tensor_copy(agr[32 * q:32 * (q + 1), :], agr[0:32, :])
            nc.scalar.activation(eag[:], agr[:], AF.Exp)
            # preload Sqrt table for the next squash (anchored on eag)
            dsq = p_small.tile([b_local, 1], F32, tag="dsq")
            nc.scalar.activation(dsq[:], eag[0:b_local, 0:1], AF.Sqrt)
            # delta = (eag-1)*e3_j keeps den incremental; then update e3
            delta = p_small.tile([128, G], F32, tag="delta")
            nc.vector.scalar_tensor_tensor(
                delta[:], eag[:], -1.0, e3[:, :, j], ALU.add, ALU.mult
            )
            nc.vector.tensor_mul(e3[:, :, j], e3[:, :, j], eag[:])
            nc.vector.tensor_add(den[:], den[:], delta[:])

    es.close()


def build_nc(n_cores=N_CORES, b_local=B // N_CORES):
    nc = bacc.Bacc(
        "TRN2", target_bir_lowering=False, debug=False,
        num_devices=n_cores,
    )
    x_dram = nc.dram_tensor("x_kd", [2, 128, G, b_local], BF16, kind="ExternalInput")
    w_dram = nc.dram_tensor("w", [J, K, D, U], BF16, kind="ExternalInput")
    out_dram = nc.dram_tensor("out", [b_local, J, U], F32, kind="ExternalOutput")
    rg = [list(range(n_cores))]
    with tile.TileContext(nc) as tc:
        capsnet_body(tc, nc, x_dram, w_dram, out_dram, rg, b_local)
    nc.compile()
    return nc


def shard_x(x_full):
    """x_full [B,1152,8,1] -> per-core [2, 128, G, b] bf16: x_a (p=khat*8+d)
    and x_b (p=d*16+khat) stacked."""
    b_local = x_full.shape[0] // N_CORES
    shards = []
    for i in range(N_CORES):
        xs = np.ascontiguousarray(
            x_full[i * b_local:(i + 1) * b_local, :, :, 0], dtype=np.float32
        )
        r = xs.reshape(b_local, G, 16, D)
        x_a = r.transpose(2, 3, 1, 0).reshape(128, G, b_local)  # khat-major
        x_b = r.transpose(3, 2, 1, 0).reshape(128, G, b_local)  # d-major
        shards.append(np.ascontiguousarray(
            np.stack([x_a, x_b]).astype(ml_dtypes.bfloat16)))
    return shards


_NC_CACHE = {}


def kernel(inputs, W, num_outputs):
    assert int(num_outputs) == J
    x_full = np.asarray(inputs, dtype=np.float32)
    w = np.ascontiguousarray(
        np.asarray(W, dtype=np.float32).astype(ml_dtypes.bfloat16))
    assert x_full.shape == (B, K, D, 1) and w.shape == (J, K, D, U)

    if "nc" not in _NC_CACHE:
        _NC_CACHE["nc"] = build_nc()
    nc = _NC_CACHE["nc"]

    shards = shard_x(x_full)
    in_maps = [{"x_kd": shards[i], "w": w} for i in range(N_CORES)]
    res = bass_utils.run_bass_kernel_spmd(
        nc, in_maps, core_ids=list(range(N_CORES))
    )
    outs = [res.results[i]["out"] for i in range(N_CORES)]  # [b, J, U] each
    full = np.concatenate(outs, axis=0)  # [B, J, U]
    return full[..., None].astype(np.float32)
